# revision 9
# baseline (speedup 1.0000x reference)
"""Multi-head self-attention (B=2, S=2048, D=1024, H=16, causal) on 8 trn2 cores.

Sharding: core c computes heads {2c, 2c+1} for both batches (column-parallel
QKV, row-parallel O). Each core returns a partial [4096, 1024] output
(attention output of its heads projected through its slice of o_proj);
the host sums the 8 partials.

Per-core kernel (all fp32 data, float32r matmuls):
  - host supplies x pre-transposed (xT [1024, 4096]) and per-core weight
    slices pre-laid-out for SBUF.
  - projections: QT/KT [128, 2048] per batch (head dims on partitions),
    V via VT + PE transpose, augmented with a ones column so the AV matmul
    also produces the softmax denominator.
  - attention in transposed-score layout: scoresT[k, q] = K @ Q^T tiles,
    exp on ACT (scale 1/8 fused), causal staircase skips invalid columns,
    triangular mask multiplies only diagonal blocks.
  - AV: avT_aug[65, q] = V_aug^T @ expT accumulated over k tiles; row 64 is
    the denominator; normalize via reciprocal + partition-broadcast DMA.
  - O: out[tok, :] = sum_h avT_h^T @ woT_h, DMA'd to DRAM.
"""

import os
import numpy as np
from contextlib import ExitStack

import concourse.bass as bass
import concourse.tile as tile
from concourse import bacc, mybir
from concourse.bass_utils import run_bass_kernel_spmd

F32R = mybir.dt.float32r
F32 = mybir.dt.float32
EXP = mybir.ActivationFunctionType.Exp

B, S, D = 2, 2048, 1024
NT = B * S            # 4096 tokens total
NCORES = 8
SCALE = 0.125         # 1/sqrt(64)

_BUILT = None
LAST_RESULTS = None


def _build():
    nc = bacc.Bacc("TRN2", target_bir_lowering=False, debug=False,
                   num_devices=NCORES)
    xt_d = nc.dram_tensor("xt", [D, NT], F32R, kind="ExternalInput").ap()
    wq_d = nc.dram_tensor("wq", [128, D], F32R, kind="ExternalInput").ap()
    wk_d = nc.dram_tensor("wk", [128, D], F32R, kind="ExternalInput").ap()
    wv_d = nc.dram_tensor("wv", [128, D], F32R, kind="ExternalInput").ap()
    wo_d = nc.dram_tensor("wo", [64, 2048], F32R, kind="ExternalInput").ap()
    tri_d = nc.dram_tensor("tri", [128, 128], F32R, kind="ExternalInput").ap()
    id_d = nc.dram_tensor("ident", [128, 128], F32R, kind="ExternalInput").ap()
    ones_d = nc.dram_tensor("ones", [128, 64], F32R, kind="ExternalInput").ap()
    out_d = nc.dram_tensor("out", [NT, D], F32R, kind="ExternalOutput").ap()

    with tile.TileContext(nc) as tc, ExitStack() as ctx:
        consts = ctx.enter_context(tc.tile_pool(name="consts", bufs=1))
        sb = ctx.enter_context(tc.tile_pool(name="sb", bufs=1))
        ps = ctx.enter_context(tc.tile_pool(name="ps", bufs=1, space="PSUM"))

        wq_t = consts.tile([128, D], F32R, tag="wq")
        nc.sync.dma_start(wq_t, wq_d)
        wk_t = consts.tile([128, D], F32R, tag="wk")
        nc.sync.dma_start(wk_t, wk_d)
        wv_t = consts.tile([128, D], F32R, tag="wv")
        nc.sync.dma_start(wv_t, wv_d)
        wo_t = consts.tile([64, 2048], F32R, tag="wo")
        nc.sync.dma_start(wo_t, wo_d)
        tri_t = consts.tile([128, 128], F32R, tag="tri")
        nc.sync.dma_start(tri_t, tri_d)
        id_t = consts.tile([128, 128], F32R, tag="ident")
        nc.sync.dma_start(id_t, id_d)
        # all-ones tile; row 64 used as lhsT of the broadcast outer-product
        ones_t = consts.tile([65, 64], F32R, tag="ones")
        nc.sync.dma_start(ones_t, ones_d[0:65, 0:64])

        for b in range(B):
            # ---------- projections ----------
            xts = []
            for k in range(8):
                xk = sb.tile([128, S], F32R, tag="xt", bufs=8)
                nc.sync.dma_start(xk, xt_d[128 * k:128 * (k + 1), S * b:S * (b + 1)])
                xts.append(xk)

            def project(w_t, tag):
                dst = sb.tile([128, S], F32R, tag=tag, bufs=2)
                for chk in range(4):
                    pp = ps.tile([128, 512], F32, tag="mm", bufs=3)
                    for k in range(8):
                        nc.tensor.matmul(
                            pp, lhsT=w_t[:, 128 * k:128 * (k + 1)],
                            rhs=xts[k][:, 512 * chk:512 * (chk + 1)],
                            start=(k == 0), stop=(k == 7))
                    nc.vector.tensor_copy(dst[:, 512 * chk:512 * (chk + 1)], pp)
                return dst

            qt = project(wq_t, "qt")
            kt = project(wk_t, "kt")
            vt = project(wv_t, "vt")

            # V in token-partition layout, + ones column for the denominator
            vg = []
            for h in range(2):
                vgh = sb.tile([128, 16, 66], F32R, tag=f"vg{h}", bufs=2)
                nc.sync.dma_start(vgh[:, :, 64:65], ones_d[:, 0:16])
                vg.append(vgh)
            for j in range(16):
                tp = ps.tile([128, 128], F32R, tag="mm", bufs=3)
                nc.tensor.transpose(tp, vt[:, 128 * j:128 * (j + 1)], id_t)
                nc.vector.tensor_copy(vg[0][:, j, 0:64], tp[:, 0:64])
                nc.vector.tensor_copy(vg[1][:, j, 0:64], tp[:, 64:128])

            # ---------- attention ----------
            for qc in range(4):
                njt = 4 * qc + 4
                avps = [ps.tile([128, 512], F32, tag="av", bufs=4,
                                name=f"avps{b}_{qc}_{h}")
                        for h in range(2)]
                pend = None  # software pipeline: AV for j-1 after scores j

                def do_av(j, ets):
                    vs = max(0, 128 * (j - 4 * qc))
                    for h in range(2):
                        nc.tensor.matmul(
                            avps[h][0:65, vs:512],
                            lhsT=vg[h][:, j, 0:65],
                            rhs=ets[h][:, vs:512],
                            start=(j == 0), stop=(j == njt - 1),
                            skip_group_check=True)

                for j in range(njt):
                    vs = max(0, 128 * (j - 4 * qc))
                    ets = []
                    for h in range(2):
                        sc = ps.tile([128, 512], F32, tag="mm", bufs=3)
                        nc.tensor.matmul(
                            sc[:, vs:512],
                            lhsT=kt[64 * h:64 * (h + 1), 128 * j:128 * (j + 1)],
                            rhs=qt[64 * h:64 * (h + 1), 512 * qc + vs:512 * (qc + 1)],
                            start=True, stop=True)
                        et = sb.tile([128, 512], F32R, tag=f"et{h}", bufs=3)
                        nc.scalar.activation(et[:, vs:512], sc[:, vs:512],
                                             EXP, scale=SCALE)
                        if j >= 4 * qc:
                            nc.vector.tensor_mul(et[:, vs:vs + 128],
                                                 et[:, vs:vs + 128], tri_t)
                        ets.append(et)
                    if pend is not None:
                        do_av(*pend)
                    pend = (j, ets)
                do_av(*pend)

                # normalize: row 64 of avps is the denominator
                avts = []
                for h in range(2):
                    rp = sb.tile([65, 512], F32R, tag=f"rp{h}", bufs=2)
                    with nc.allow_low_precision(reason="f32r is f32 bits"):
                        nc.vector.reciprocal(rp[64:65, :], avps[h][64:65, :])
                    # broadcast recip row across 64 partitions via outer
                    # product with a ones column (both operands at part. 64)
                    sclps = ps.tile([64, 512], F32, tag="mm", bufs=3,
                                    name=f"sclps{b}_{qc}_{h}")
                    nc.tensor.matmul(sclps, lhsT=ones_t[64:65, :],
                                     rhs=rp[64:65, :], start=True, stop=True)
                    scl = sb.tile([64, 512], F32, tag=f"scl{h}", bufs=2)
                    nc.vector.tensor_copy(scl, sclps)
                    avt = sb.tile([64, 512], F32R, tag=f"avt{h}", bufs=2)
                    nc.vector.tensor_mul(avt, avps[h][0:64, :], scl)
                    avts.append(avt)

                # O projection (contraction 64 per head, accumulate heads)
                for tt in range(4):
                    ost = sb.tile([128, 1024], F32R, tag="ost", bufs=2)
                    for chv in range(2):
                        op = ps.tile([128, 512], F32, tag="mm", bufs=3)
                        for h in range(2):
                            nc.tensor.matmul(
                                op,
                                lhsT=avts[h][:, 128 * tt:128 * (tt + 1)],
                                rhs=wo_t[:, 1024 * h + 512 * chv:
                                         1024 * h + 512 * (chv + 1)],
                                start=(h == 0), stop=(h == 1))
                        nc.vector.tensor_copy(ost[:, 512 * chv:512 * (chv + 1)], op)
                    row0 = S * b + 512 * qc + 128 * tt
                    nc.sync.dma_start(out_d[row0:row0 + 128, :], ost)
    nc.compile()
    return nc


def _get_built():
    global _BUILT
    if _BUILT is None:
        _BUILT = _build()
    return _BUILT


def _host_inputs(x, q_proj, k_proj, v_proj, o_proj):
    xth = np.ascontiguousarray(x.reshape(NT, D).T)
    tri = np.triu(np.ones((128, 128), dtype=np.float32))
    ident = np.eye(128, dtype=np.float32)

    def wslice(w, c):
        # [p, 8k x 128m]: w_sb[p, 128k+m] = w[128c+m, 128k+p]
        a = w[128 * c:128 * (c + 1)].reshape(128, 8, 128)
        return np.ascontiguousarray(a.transpose(2, 1, 0).reshape(128, D))

    in_maps = []
    for c in range(NCORES):
        wo = o_proj[:, 128 * c:128 * (c + 1)].reshape(D, 2, 64)
        wo = np.ascontiguousarray(wo.transpose(2, 1, 0).reshape(64, 2048))
        in_maps.append(dict(
            xt=xth, wq=wslice(q_proj, c), wk=wslice(k_proj, c),
            wv=wslice(v_proj, c), wo=wo, tri=tri, ident=ident,
            ones=np.ones((128, 64), dtype=np.float32)))
    return in_maps


def kernel(**inputs):
    x = np.asarray(inputs["x"], dtype=np.float32)
    q_proj = np.asarray(inputs["q_proj"], dtype=np.float32)
    k_proj = np.asarray(inputs["k_proj"], dtype=np.float32)
    v_proj = np.asarray(inputs["v_proj"], dtype=np.float32)
    o_proj = np.asarray(inputs["o_proj"], dtype=np.float32)

    in_maps = _host_inputs(x, q_proj, k_proj, v_proj, o_proj)
    nc = _get_built()
    global LAST_RESULTS
    LAST_RESULTS = run_bass_kernel_spmd(
        nc, in_maps, core_ids=list(range(NCORES)),
        trace=bool(os.environ.get("KERNEL_TRACE")))
    acc = LAST_RESULTS.results[0]["out"].astype(np.float32).copy()
    for c in range(1, NCORES):
        acc += LAST_RESULTS.results[c]["out"]
    return acc.reshape(B, S, D)


# revision 11
# speedup vs baseline: 1.2005x; 1.2005x over previous
"""Multi-head self-attention (B=2, S=2048, D=1024, H=16, causal) on 8 trn2 cores.

Sharding: core c computes heads {2c, 2c+1} for both batches (column-parallel
QKV, row-parallel O). Each core returns a partial [4096, 1024] output
(attention output of its heads projected through its slice of o_proj);
the host sums the 8 partials.

Per-core kernel:
  - host supplies x pre-transposed (xT [1024, 4096]) and per-core weight
    slices pre-laid-out for SBUF.
  - projections (f32r matmuls): QT/KT [128, 2048] per batch stored bf16
    (head dims on partitions), V via VT + PE transpose kept f32r,
    augmented with a ones column so the AV matmul also produces the
    softmax denominator.
  - attention in transposed-score layout: scoresT[k, q] = K @ Q^T tiles
    (bf16, two heads packed on PE row groups), exp on ACT (scale 1/8
    fused), causal staircase skips invalid columns, triangular mask
    multiplies only diagonal blocks.
  - AV (f32r): avT_aug[65, q] = V_aug^T @ expT accumulated over k tiles;
    row 64 is the denominator.
  - normalize without reciprocal: ln(denom) on ACT -> broadcast across 64
    partitions via fp32 ones-outer-product matmul -> exp(-x) on ACT ->
    multiply. Head 1's raw output is shifted to partitions 64:128 with an
    SBUF->SBUF DMA so O can contract over all 128 dims in one chain.
  - O (f32r, K=128): out[tok, :] = avT_all^T @ woT, DMA'd to DRAM.
"""

import os
import numpy as np
from contextlib import ExitStack

import concourse.bass as bass
import concourse.tile as tile
from concourse import bacc, mybir
from concourse.bass_utils import run_bass_kernel_spmd

F32R = mybir.dt.float32r
F32 = mybir.dt.float32
BF16 = mybir.dt.bfloat16
EXP = mybir.ActivationFunctionType.Exp
LN = mybir.ActivationFunctionType.Ln

B, S, D = 2, 2048, 1024
NT = B * S            # 4096 tokens total
NCORES = 8
SCALE = 0.125         # 1/sqrt(64)

_BUILT = None
LAST_RESULTS = None


def _build():
    nc = bacc.Bacc("TRN2", target_bir_lowering=False, debug=False,
                   num_devices=NCORES)
    xt_d = nc.dram_tensor("xt", [D, NT], F32R, kind="ExternalInput").ap()
    wq_d = nc.dram_tensor("wq", [128, D], F32R, kind="ExternalInput").ap()
    wk_d = nc.dram_tensor("wk", [128, D], F32R, kind="ExternalInput").ap()
    wv_d = nc.dram_tensor("wv", [128, D], F32R, kind="ExternalInput").ap()
    wo_d = nc.dram_tensor("wo", [128, 1024], F32R, kind="ExternalInput").ap()
    tri_d = nc.dram_tensor("tri", [128, 128], F32R, kind="ExternalInput").ap()
    id_d = nc.dram_tensor("ident", [128, 128], F32R, kind="ExternalInput").ap()
    ones_d = nc.dram_tensor("ones", [128, 64], F32, kind="ExternalInput").ap()
    out_d = nc.dram_tensor("out", [NT, D], F32R, kind="ExternalOutput").ap()

    with tile.TileContext(nc) as tc, ExitStack() as ctx:
        consts = ctx.enter_context(tc.tile_pool(name="consts", bufs=1))
        sb = ctx.enter_context(tc.tile_pool(name="sb", bufs=1))
        ps = ctx.enter_context(tc.tile_pool(name="ps", bufs=1, space="PSUM"))

        wq_t = consts.tile([128, D], F32R, tag="wq")
        nc.sync.dma_start(wq_t, wq_d)
        wk_t = consts.tile([128, D], F32R, tag="wk")
        nc.sync.dma_start(wk_t, wk_d)
        wv_t = consts.tile([128, D], F32R, tag="wv")
        nc.sync.dma_start(wv_t, wv_d)
        wo_t = consts.tile([128, 1024], F32R, tag="wo")
        nc.sync.dma_start(wo_t, wo_d)
        tri_t = consts.tile([128, 128], F32R, tag="tri")
        nc.sync.dma_start(tri_t, tri_d)
        id_t = consts.tile([128, 128], F32R, tag="ident")
        nc.sync.dma_start(id_t, id_d)
        # all-ones; row 64 is the lhsT of the fp32 broadcast outer-product
        ones_t = consts.tile([65, 64], F32, tag="ones")
        nc.sync.dma_start(ones_t, ones_d[0:65, 0:64])

        for b in range(B):
            # ---------- projections ----------
            xts = []
            for k in range(8):
                xk = sb.tile([128, S], F32R, tag="xt", bufs=8)
                nc.sync.dma_start(xk, xt_d[128 * k:128 * (k + 1), S * b:S * (b + 1)])
                xts.append(xk)

            def project(w_t, tag, dt):
                dst = sb.tile([128, S], dt, tag=tag, bufs=2)
                for chk in range(4):
                    pp = ps.tile([128, 512], F32, tag="mm", bufs=3)
                    for k in range(8):
                        nc.tensor.matmul(
                            pp, lhsT=w_t[:, 128 * k:128 * (k + 1)],
                            rhs=xts[k][:, 512 * chk:512 * (chk + 1)],
                            start=(k == 0), stop=(k == 7))
                    nc.vector.tensor_copy(dst[:, 512 * chk:512 * (chk + 1)], pp)
                return dst

            qt = project(wq_t, "qt", BF16)
            kt = project(wk_t, "kt", BF16)
            vt = project(wv_t, "vt", F32R)

            # V in token-partition layout, + ones column for the denominator
            vg = []
            for h in range(2):
                vgh = sb.tile([128, 16, 66], F32R, tag=f"vg{h}", bufs=2)
                # gpsimd DMA: casts F32 ones -> F32R (bit-identical)
                nc.gpsimd.dma_start(vgh[:, :, 64:65], ones_d[:, 0:16])
                vg.append(vgh)
            for j in range(16):
                tp = ps.tile([128, 128], F32R, tag="mm", bufs=3)
                nc.tensor.transpose(tp, vt[:, 128 * j:128 * (j + 1)], id_t)
                nc.vector.tensor_copy(vg[0][:, j, 0:64], tp[:, 0:64])
                nc.vector.tensor_copy(vg[1][:, j, 0:64], tp[:, 64:128])

            # ---------- attention ----------
            for qc in range(4):
                njt = 4 * qc + 4
                avps = [ps.tile([128, 512], F32, tag="av", bufs=4,
                                name=f"avps{b}_{qc}_{h}")
                        for h in range(2)]
                pend = None  # software pipeline: AV for j-1 after scores j

                def do_av(j, ets):
                    vs = max(0, 128 * (j - 4 * qc))
                    for h in range(2):
                        nc.tensor.matmul(
                            avps[h][0:65, vs:512],
                            lhsT=vg[h][:, j, 0:65],
                            rhs=ets[h][:, vs:512],
                            start=(j == 0), stop=(j == njt - 1),
                            skip_group_check=True)

                for j in range(njt):
                    vs = max(0, 128 * (j - 4 * qc))
                    ets = []
                    for h in range(2):
                        sc = ps.tile([128, 512], F32, tag="mm", bufs=3)
                        nc.tensor.matmul(
                            sc[:, vs:512],
                            lhsT=kt[64 * h:64 * (h + 1), 128 * j:128 * (j + 1)],
                            rhs=qt[64 * h:64 * (h + 1), 512 * qc + vs:512 * (qc + 1)],
                            start=True, stop=True)
                        et = sb.tile([128, 512], F32R, tag=f"et{h}", bufs=3)
                        nc.scalar.activation(et[:, vs:512], sc[:, vs:512],
                                             EXP, scale=SCALE)
                        if j >= 4 * qc:
                            nc.vector.tensor_mul(et[:, vs:vs + 128],
                                                 et[:, vs:vs + 128], tri_t)
                        ets.append(et)
                    if pend is not None:
                        do_av(*pend)
                    pend = (j, ets)
                do_av(*pend)

                # normalize: row 64 of avps holds the denominator.
                # r = exp(-ln(denom)) broadcast across partitions via fp32
                # ones-outer-product matmul (no DVE reciprocal).
                avt_all = sb.tile([128, 512], F32R, tag="avt", bufs=2)
                scl = sb.tile([128, 512], F32R, tag="scl", bufs=2)
                lnr = sb.tile([65, 512], F32, tag="lnr", bufs=2)
                # h1 raw unnormalized values -> partitions 64:128 via DMA
                stg = sb.tile([64, 512], F32R, tag="stg", bufs=2)
                rawsh = sb.tile([128, 512], F32R, tag="rawsh", bufs=2)
                nc.vector.tensor_copy(stg, avps[1][0:64, :])
                nc.sync.dma_start(rawsh[64:128, :], stg)

                nc.scalar.activation(lnr[64:65, :], avps[0][64:65, :], LN)
                bc0 = ps.tile([64, 512], F32, tag="mm", bufs=3,
                              name=f"bc0_{b}_{qc}")
                nc.tensor.matmul(bc0, lhsT=ones_t[64:65, :],
                                 rhs=lnr[64:65, :], start=True, stop=True)
                nc.scalar.activation(scl[0:64, :], bc0, EXP, scale=-1.0)
                nc.vector.tensor_mul(avt_all[0:64, :], avps[0][0:64, :],
                                     scl[0:64, :])

                lnr2 = sb.tile([65, 512], F32, tag="lnr2", bufs=2)
                nc.scalar.activation(lnr2[64:65, :], avps[1][64:65, :], LN)
                bc1 = ps.tile([128, 512], F32, tag="mm", bufs=3,
                              name=f"bc1_{b}_{qc}")
                nc.tensor.matmul(bc1[64:128, :], lhsT=ones_t[64:65, :],
                                 rhs=lnr2[64:65, :], start=True, stop=True)
                nc.scalar.activation(scl[64:128, :], bc1[64:128, :], EXP,
                                     scale=-1.0)
                nc.vector.tensor_mul(avt_all[64:128, :], rawsh[64:128, :],
                                     scl[64:128, :])

                # O projection: contraction over all 128 dims in one chain
                for tt in range(4):
                    ost = sb.tile([128, 1024], F32R, tag="ost", bufs=2)
                    for chv in range(2):
                        op = ps.tile([128, 512], F32, tag="mm", bufs=3)
                        nc.tensor.matmul(
                            op,
                            lhsT=avt_all[:, 128 * tt:128 * (tt + 1)],
                            rhs=wo_t[:, 512 * chv:512 * (chv + 1)],
                            start=True, stop=True)
                        nc.vector.tensor_copy(ost[:, 512 * chv:512 * (chv + 1)], op)
                    row0 = S * b + 512 * qc + 128 * tt
                    nc.sync.dma_start(out_d[row0:row0 + 128, :], ost)
    nc.compile()
    return nc


def _get_built():
    global _BUILT
    if _BUILT is None:
        _BUILT = _build()
    return _BUILT


def _host_inputs(x, q_proj, k_proj, v_proj, o_proj):
    xth = np.ascontiguousarray(x.reshape(NT, D).T)
    tri = np.triu(np.ones((128, 128), dtype=np.float32))
    ident = np.eye(128, dtype=np.float32)

    def wslice(w, c):
        # [p, 8k x 128m]: w_sb[p, 128k+m] = w[128c+m, 128k+p]
        a = w[128 * c:128 * (c + 1)].reshape(128, 8, 128)
        return np.ascontiguousarray(a.transpose(2, 1, 0).reshape(128, D))

    in_maps = []
    for c in range(NCORES):
        wo = np.ascontiguousarray(o_proj[:, 128 * c:128 * (c + 1)].T)
        in_maps.append(dict(
            xt=xth, wq=wslice(q_proj, c), wk=wslice(k_proj, c),
            wv=wslice(v_proj, c), wo=wo, tri=tri, ident=ident,
            ones=np.ones((128, 64), dtype=np.float32)))
    return in_maps


def kernel(**inputs):
    x = np.asarray(inputs["x"], dtype=np.float32)
    q_proj = np.asarray(inputs["q_proj"], dtype=np.float32)
    k_proj = np.asarray(inputs["k_proj"], dtype=np.float32)
    v_proj = np.asarray(inputs["v_proj"], dtype=np.float32)
    o_proj = np.asarray(inputs["o_proj"], dtype=np.float32)

    in_maps = _host_inputs(x, q_proj, k_proj, v_proj, o_proj)
    nc = _get_built()
    global LAST_RESULTS
    LAST_RESULTS = run_bass_kernel_spmd(
        nc, in_maps, core_ids=list(range(NCORES)),
        trace=bool(os.environ.get("KERNEL_TRACE")))
    acc = LAST_RESULTS.results[0]["out"].astype(np.float32).copy()
    for c in range(1, NCORES):
        acc += LAST_RESULTS.results[c]["out"]
    return acc.reshape(B, S, D)


# revision 13
# speedup vs baseline: 1.2145x; 1.0117x over previous
"""Multi-head self-attention (B=2, S=2048, D=1024, H=16, causal) on 8 trn2 cores.

Sharding: core c computes heads {2c, 2c+1} for both batches (column-parallel
QKV, row-parallel O). Each core returns a partial [4096, 1024] output
(attention output of its heads projected through its slice of o_proj);
the host sums the 8 partials.

Per-core kernel:
  - host supplies x pre-transposed (xT [1024, 4096]) and per-core weight
    slices pre-laid-out for SBUF.
  - projections (f32r matmuls): QT/KT [128, 2048] per batch stored bf16
    (head dims on partitions), V via VT + PE transpose kept f32r,
    augmented with a ones column so the AV matmul also produces the
    softmax denominator.
  - attention in transposed-score layout: scoresT[k, q] = K @ Q^T tiles
    (bf16, two heads packed on PE row groups), exp on ACT (scale 1/8
    fused), causal staircase skips invalid columns, triangular mask
    multiplies only diagonal blocks.
  - AV (f32r): avT_aug[65, q] = V_aug^T @ expT accumulated over k tiles;
    row 64 is the denominator.
  - normalize without reciprocal: ln(denom) on ACT -> broadcast across 64
    partitions via fp32 ones-outer-product matmul -> exp(-x) on ACT ->
    multiply. Head 1's raw output is shifted to partitions 64:128 with an
    SBUF->SBUF DMA so O can contract over all 128 dims in one chain.
  - O (f32r, K=128): out[tok, :] = avT_all^T @ woT, DMA'd to DRAM.
"""

import os
import numpy as np
from contextlib import ExitStack

import concourse.bass as bass
import concourse.tile as tile
from concourse import bacc, mybir
from concourse.bass_utils import run_bass_kernel_spmd

F32R = mybir.dt.float32r
F32 = mybir.dt.float32
BF16 = mybir.dt.bfloat16
EXP = mybir.ActivationFunctionType.Exp
LN = mybir.ActivationFunctionType.Ln

B, S, D = 2, 2048, 1024
NT = B * S            # 4096 tokens total
NCORES = 8
SCALE = 0.125         # 1/sqrt(64)

_BUILT = None
LAST_RESULTS = None


def _build():
    nc = bacc.Bacc("TRN2", target_bir_lowering=False, debug=False,
                   num_devices=NCORES)
    xt_d = nc.dram_tensor("xt", [D, NT], F32R, kind="ExternalInput").ap()
    wq_d = nc.dram_tensor("wq", [128, D], F32R, kind="ExternalInput").ap()
    wk_d = nc.dram_tensor("wk", [128, D], F32R, kind="ExternalInput").ap()
    wv_d = nc.dram_tensor("wv", [128, D], F32R, kind="ExternalInput").ap()
    wo_d = nc.dram_tensor("wo", [128, 1024], F32R, kind="ExternalInput").ap()
    tri_d = nc.dram_tensor("tri", [128, 128], F32R, kind="ExternalInput").ap()
    id_d = nc.dram_tensor("ident", [128, 128], F32R, kind="ExternalInput").ap()
    ones_d = nc.dram_tensor("ones", [128, 64], F32, kind="ExternalInput").ap()
    out_d = nc.dram_tensor("out", [NT, D], F32R, kind="ExternalOutput").ap()

    with tile.TileContext(nc) as tc, ExitStack() as ctx:
        consts = ctx.enter_context(tc.tile_pool(name="consts", bufs=1))
        sb = ctx.enter_context(tc.tile_pool(name="sb", bufs=1))
        ps = ctx.enter_context(tc.tile_pool(name="ps", bufs=1, space="PSUM"))

        wq_t = consts.tile([128, D], F32R, tag="wq")
        nc.sync.dma_start(wq_t, wq_d)
        wk_t = consts.tile([128, D], F32R, tag="wk")
        nc.sync.dma_start(wk_t, wk_d)
        wv_t = consts.tile([128, D], F32R, tag="wv")
        nc.sync.dma_start(wv_t, wv_d)
        wo_t = consts.tile([128, 1024], F32R, tag="wo")
        nc.sync.dma_start(wo_t, wo_d)
        tri_t = consts.tile([128, 128], F32R, tag="tri")
        nc.sync.dma_start(tri_t, tri_d)
        id_t = consts.tile([128, 128], F32R, tag="ident")
        nc.sync.dma_start(id_t, id_d)
        # all-ones; row 64 is the lhsT of the fp32 broadcast outer-product
        ones_t = consts.tile([65, 64], F32, tag="ones")
        nc.sync.dma_start(ones_t, ones_d[0:65, 0:64])

        def normalize_and_output(b, qc, avps):
            # normalize: row 64 of avps holds the denominator.
            # r = exp(-ln(denom)) broadcast across partitions via fp32
            # ones-outer-product matmul (no DVE reciprocal).
            avt_all = sb.tile([128, 512], F32R, tag="avt", bufs=2,
                              name=f"avt{b}_{qc}")
            scl = sb.tile([128, 512], F32R, tag="scl", bufs=2,
                          name=f"scl{b}_{qc}")
            lnr = sb.tile([65, 512], F32, tag="lnr", bufs=2,
                          name=f"lnr{b}_{qc}")
            lnr2 = sb.tile([65, 512], F32, tag="lnr2", bufs=2,
                           name=f"lnr2{b}_{qc}")
            # h1 raw unnormalized values -> partitions 64:128 via DMA
            stg = sb.tile([64, 512], F32R, tag="stg", bufs=2,
                          name=f"stg{b}_{qc}")
            rawsh = sb.tile([128, 512], F32R, tag="rawsh", bufs=2,
                            name=f"rawsh{b}_{qc}")
            nc.vector.tensor_copy(stg, avps[1][0:64, :])
            nc.sync.dma_start(rawsh[64:128, :], stg)

            nc.scalar.activation(lnr[64:65, :], avps[0][64:65, :], LN)
            nc.scalar.activation(lnr2[64:65, :], avps[1][64:65, :], LN)
            bc0 = ps.tile([64, 512], F32, tag="mm", bufs=3,
                          name=f"bc0_{b}_{qc}")
            nc.tensor.matmul(bc0, lhsT=ones_t[64:65, :],
                             rhs=lnr[64:65, :], start=True, stop=True)
            bc1 = ps.tile([128, 512], F32, tag="mm", bufs=3,
                          name=f"bc1_{b}_{qc}")
            nc.tensor.matmul(bc1[64:128, :], lhsT=ones_t[64:65, :],
                             rhs=lnr2[64:65, :], start=True, stop=True)
            nc.scalar.activation(scl[0:64, :], bc0, EXP, scale=-1.0)
            nc.scalar.activation(scl[64:128, :], bc1[64:128, :], EXP,
                                 scale=-1.0)
            nc.vector.tensor_mul(avt_all[0:64, :], avps[0][0:64, :],
                                 scl[0:64, :])
            nc.vector.tensor_mul(avt_all[64:128, :], rawsh[64:128, :],
                                 scl[64:128, :])

            # O projection: contraction over all 128 dims in one chain
            for tt in range(4):
                ost = sb.tile([128, 1024], F32R, tag="ost", bufs=2,
                              name=f"ost{b}_{qc}_{tt}")
                for chv in range(2):
                    op = ps.tile([128, 512], F32, tag="mm", bufs=3,
                                 name=f"op{b}_{qc}_{tt}_{chv}")
                    nc.tensor.matmul(
                        op,
                        lhsT=avt_all[:, 128 * tt:128 * (tt + 1)],
                        rhs=wo_t[:, 512 * chv:512 * (chv + 1)],
                        start=True, stop=True)
                    nc.vector.tensor_copy(ost[:, 512 * chv:512 * (chv + 1)], op)
                row0 = S * b + 512 * qc + 128 * tt
                nc.sync.dma_start(out_d[row0:row0 + 128, :], ost)

        pending_no = None  # (b, qc, avps): normalize+O deferred one qc
        for b in range(B):
            # ---------- projections ----------
            xts = []
            for k in range(8):
                xk = sb.tile([128, S], F32R, tag="xt", bufs=8)
                nc.sync.dma_start(xk, xt_d[128 * k:128 * (k + 1), S * b:S * (b + 1)])
                xts.append(xk)

            def project(w_t, tag, dt):
                dst = sb.tile([128, S], dt, tag=tag, bufs=2)
                for chk in range(4):
                    pp = ps.tile([128, 512], F32, tag="mm", bufs=3)
                    for k in range(8):
                        nc.tensor.matmul(
                            pp, lhsT=w_t[:, 128 * k:128 * (k + 1)],
                            rhs=xts[k][:, 512 * chk:512 * (chk + 1)],
                            start=(k == 0), stop=(k == 7))
                    nc.vector.tensor_copy(dst[:, 512 * chk:512 * (chk + 1)], pp)
                return dst

            qt = project(wq_t, "qt", BF16)
            kt = project(wk_t, "kt", BF16)
            vt = project(wv_t, "vt", F32R)

            # V in token-partition layout, + ones column for the denominator
            vg = []
            for h in range(2):
                vgh = sb.tile([128, 16, 66], F32R, tag=f"vg{h}", bufs=2)
                # gpsimd DMA: casts F32 ones -> F32R (bit-identical)
                nc.gpsimd.dma_start(vgh[:, :, 64:65], ones_d[:, 0:16])
                vg.append(vgh)
            for j in range(16):
                tp = ps.tile([128, 128], F32R, tag="mm", bufs=3)
                nc.tensor.transpose(tp, vt[:, 128 * j:128 * (j + 1)], id_t)
                nc.vector.tensor_copy(vg[0][:, j, 0:64], tp[:, 0:64])
                nc.vector.tensor_copy(vg[1][:, j, 0:64], tp[:, 64:128])

            # ---------- attention ----------
            for qc in range(4):
                njt = 4 * qc + 4
                avps = [ps.tile([128, 512], F32, tag="av", bufs=4,
                                name=f"avps{b}_{qc}_{h}")
                        for h in range(2)]
                pend = None  # software pipeline: AV for j-1 after scores j

                def do_av(j, ets):
                    vs = max(0, 128 * (j - 4 * qc))
                    for h in range(2):
                        nc.tensor.matmul(
                            avps[h][0:65, vs:512],
                            lhsT=vg[h][:, j, 0:65],
                            rhs=ets[h][:, vs:512],
                            start=(j == 0), stop=(j == njt - 1),
                            skip_group_check=True)

                for j in range(njt):
                    vs = max(0, 128 * (j - 4 * qc))
                    ets = []
                    for h in range(2):
                        sc = ps.tile([128, 512], F32, tag="mm", bufs=3)
                        nc.tensor.matmul(
                            sc[:, vs:512],
                            lhsT=kt[64 * h:64 * (h + 1), 128 * j:128 * (j + 1)],
                            rhs=qt[64 * h:64 * (h + 1), 512 * qc + vs:512 * (qc + 1)],
                            start=True, stop=True)
                        et = sb.tile([128, 512], F32R, tag=f"et{h}", bufs=3)
                        nc.scalar.activation(et[:, vs:512], sc[:, vs:512],
                                             EXP, scale=SCALE)
                        if j >= 4 * qc:
                            nc.vector.tensor_mul(et[:, vs:vs + 128],
                                                 et[:, vs:vs + 128], tri_t)
                        ets.append(et)
                    if pend is not None:
                        do_av(*pend)
                    pend = (j, ets)
                    if j == 1 and pending_no is not None:
                        # deferred normalize+O of the previous qc, emitted
                        # inside this qc's score stream so PE stays dense
                        normalize_and_output(*pending_no)
                        pending_no = None
                do_av(*pend)
                if pending_no is not None:  # qc had <2 j iters to hide it
                    normalize_and_output(*pending_no)
                pending_no = (b, qc, avps)
        normalize_and_output(*pending_no)
    nc.compile()
    return nc


def _get_built():
    global _BUILT
    if _BUILT is None:
        _BUILT = _build()
    return _BUILT


def _host_inputs(x, q_proj, k_proj, v_proj, o_proj):
    xth = np.ascontiguousarray(x.reshape(NT, D).T)
    tri = np.triu(np.ones((128, 128), dtype=np.float32))
    ident = np.eye(128, dtype=np.float32)

    def wslice(w, c):
        # [p, 8k x 128m]: w_sb[p, 128k+m] = w[128c+m, 128k+p]
        a = w[128 * c:128 * (c + 1)].reshape(128, 8, 128)
        return np.ascontiguousarray(a.transpose(2, 1, 0).reshape(128, D))

    in_maps = []
    for c in range(NCORES):
        wo = np.ascontiguousarray(o_proj[:, 128 * c:128 * (c + 1)].T)
        in_maps.append(dict(
            xt=xth, wq=wslice(q_proj, c), wk=wslice(k_proj, c),
            wv=wslice(v_proj, c), wo=wo, tri=tri, ident=ident,
            ones=np.ones((128, 64), dtype=np.float32)))
    return in_maps


def kernel(**inputs):
    x = np.asarray(inputs["x"], dtype=np.float32)
    q_proj = np.asarray(inputs["q_proj"], dtype=np.float32)
    k_proj = np.asarray(inputs["k_proj"], dtype=np.float32)
    v_proj = np.asarray(inputs["v_proj"], dtype=np.float32)
    o_proj = np.asarray(inputs["o_proj"], dtype=np.float32)

    in_maps = _host_inputs(x, q_proj, k_proj, v_proj, o_proj)
    nc = _get_built()
    global LAST_RESULTS
    LAST_RESULTS = run_bass_kernel_spmd(
        nc, in_maps, core_ids=list(range(NCORES)),
        trace=bool(os.environ.get("KERNEL_TRACE")))
    acc = LAST_RESULTS.results[0]["out"].astype(np.float32).copy()
    for c in range(1, NCORES):
        acc += LAST_RESULTS.results[c]["out"]
    return acc.reshape(B, S, D)
